# revision 1
# baseline (speedup 1.0000x reference)
"""CNF GNN message-passing layer on 8 Trainium2 NeuronCores (Bass/Tile).

Strategy (edge/graph parallel, clause-owner sharding):
  - Core k owns clause range [k*CPC, (k+1)*CPC) and processes exactly the
    edges whose clause falls in its range (~1/8 of edges), for BOTH message
    passing directions.
  - Phase 1 (l2c): gather raw lit_feat rows per edge (hardware dma_gather,
    int16 indices -> lit table split into <=32768-row chunks), segment-sum
    into per-clause-window PSUM accumulators via one-hot membership matmuls,
    then apply mean + W_l2c/b_l2c projection + relu + the [cembs|clause_feat]
    @ W_c2l + b_c2l projection entirely on-chip, producing the local slice of
    the Wh_c2l message table.  (segment_mean commutes with the linear layer:
    mean(Wh[src]) = mean(feat[src]) @ W + has_deg * b.)
  - Phase 2 (c2l): gather Wh_c2l rows from the LOCAL table slice per edge,
    segment-sum into full-range lit windows -> partial table T_k, then a
    ReduceScatter(add) across the 8 cores sums partials and hands each core
    its lit slice, which is finalized with mean (1/deg scale) + relu.
  - Degrees / reciprocals are index-only preprocessing, computed on host.

All per-core variation is carried in input data (index streams, membership
rel-ids, aux rows) so one SPMD program serves all 8 cores.
"""
import sys
sys.path.insert(0, "/opt/trn_rl_repo")

import math
import numpy as np

P = 128           # partitions / tile edge
D = 128           # feature width (all of IN/CLAUSE/OUT sizes)
NCORES = 8
WG = 8            # windows per gather-group
CHUNK_MAX = 32768  # int16 index range for dma_gather
NQ = 4            # SWDGE queues for gather descriptor generation

F16 = True        # table/stream dtype: float16 (False -> float32)


# ----------------------------------------------------------------------------
# host-side graph preprocessing
# ----------------------------------------------------------------------------

def _ceil_to(x, m):
    return (x + m - 1) // m * m


def _chunk_bounds(nrows):
    """Split [0, nrows) into chunks of <= CHUNK_MAX rows."""
    nch = max(1, math.ceil(nrows / CHUNK_MAX))
    bounds = [min(i * CHUNK_MAX, nrows) for i in range(nch + 1)]
    bounds[-1] = nrows
    return bounds


def _build_streams(dst_local, src, vals, n_win, bounds, quotas, n_group):
    """Build gather-idx / membership-rel streams for one core & one phase.

    dst_local: per-edge destination-window-local id  (win*128 + rel)
    src:       per-edge source row id (into the gather table)
    vals:      optional per-edge scale (e.g. 1/deg of destination); scattered
               into a per-slot array (pads = 0)
    n_win:     number of 128-row destination windows (padded to n_group*WG)
    bounds:    chunk boundaries over the source-row space
    quotas:    tiles (128-slot groups) per (window, chunk)
    """
    nch = len(bounds) - 1
    ncols = 8 * sum(quotas) // 8  # columns per group block = sum over chunks of WG*q... computed below
    # staging column layout within a group block:
    #   chunk c occupies cols [colbase[c], colbase[c] + WG*quotas[c])
    #   window w (group-local), tile t -> col colbase[c] + w*quotas[c] + t
    colbase = []
    acc = 0
    for c in range(nch):
        colbase.append(acc)
        acc += WG * quotas[c]
    ncols = acc

    win = dst_local >> 7
    rel = dst_local & 127
    chunk = np.searchsorted(bounds, src, side="right") - 1
    # order edges by (window, chunk)
    key = win * nch + chunk
    order = np.argsort(key, kind="stable")
    key_s = key[order]
    src_s = src[order]
    rel_s = rel[order]
    chunk_s = chunk[order]
    win_s = win[order]
    # rank within each (window, chunk) run
    starts = np.searchsorted(key_s, np.arange(n_win * nch))
    run_start = starts[key_s]
    rank = np.arange(len(key_s)) - run_start
    counts = np.bincount(key_s, minlength=n_win * nch).reshape(n_win, nch)
    for c in range(nch):
        assert counts[:, c].max(initial=0) <= quotas[c] * P, (
            f"chunk {c} count {counts[:, c].max()} exceeds quota {quotas[c] * P}")

    # slot address: group g, col (within group block), partition p
    g = win_s // WG
    wl = win_s % WG
    col = np.array(colbase)[chunk_s] + wl * np.array(quotas)[chunk_s] + (rank >> 7)
    p = rank & 127
    flatcol = g * ncols + col

    n_group_cols = n_group * ncols
    rel_arr = np.full((P, n_group_cols), -1.0, np.float16 if F16 else np.float32)
    rel_arr[p, flatcol] = rel_s.astype(rel_arr.dtype)
    val_arr = None
    if vals is not None:
        val_arr = np.zeros((P, n_group_cols), rel_arr.dtype)
        val_arr[p, flatcol] = vals[order].astype(rel_arr.dtype)

    # idx streams per chunk: call for (group, chunk) covers WG*quotas[c]*128 slots,
    # enumerated col-major (slot i = col_local*128 + p)
    idx_streams = []
    for c in range(nch):
        qc = quotas[c]
        ncall = WG * qc * P               # idx per call
        arr = np.zeros((n_group, ncall), np.int16)
        m = chunk_s == c
        # call-local position: (wl*qc + tile)*128 + p  == (col - colbase[c])*128 + p
        pos = (col[m] - colbase[c]) * P + p[m]
        arr[g[m], pos] = (src_s[m] - bounds[c]).astype(np.int16)
        # wrap into 16 partitions, replicate x8 -> [128, n_group*ncall/16]
        w = arr.reshape(n_group, ncall // 16, 16).transpose(2, 0, 1).reshape(16, -1)
        idx_streams.append(np.tile(w, (8, 1)).copy())
    return idx_streams, rel_arr, val_arr, ncols, colbase


def _prep(inputs):
    """All host preprocessing. Returns (meta, in_maps)."""
    lit_feat = np.asarray(inputs["lit_feat"], np.float32)
    clause_feat = np.asarray(inputs["clause_feat"], np.float32)
    el = np.asarray(inputs["edge_lit"]).astype(np.int64)
    ec = np.asarray(inputs["edge_clause"]).astype(np.int64)
    W_l2c = np.asarray(inputs["W_l2c"], np.float32)
    b_l2c = np.asarray(inputs["b_l2c"], np.float32)
    W_c2l = np.asarray(inputs["W_c2l"], np.float32)
    b_c2l = np.asarray(inputs["b_c2l"], np.float32)

    n_lit = lit_feat.shape[0]
    n_clause = clause_feat.shape[0]
    tdt = np.float16 if F16 else np.float32

    CPC = n_clause // NCORES                       # clauses per core
    NWIN1 = _ceil_to(_ceil_to(CPC, P) // P, WG)    # clause windows per core (padded)
    NG1 = NWIN1 // WG
    CLROWS = NWIN1 * P                             # padded clause rows per core

    LITROWS = _ceil_to(n_lit, P)
    NWIN2 = _ceil_to(LITROWS // P, WG)             # lit windows (full range, padded)
    NG2 = NWIN2 // WG
    TROWS = NWIN2 * P                              # T table rows (div by 8*... )
    assert TROWS % NCORES == 0
    SLICE = TROWS // NCORES                        # rows per core post-RS
    NW3 = SLICE // P                               # finalize windows per core

    # degrees (global, index-only)
    degc = np.bincount(ec, minlength=n_clause).astype(np.float32)
    degl = np.bincount(el, minlength=n_lit).astype(np.float32)
    recipc = 1.0 / np.maximum(degc, 1.0)
    hasc = (degc > 0).astype(np.float32)

    owner = ec // CPC
    # phase-1 source chunking over lit rows
    b1 = _chunk_bounds(n_lit)
    # phase-2 source chunking over local clause table rows
    b2 = _chunk_bounds(CLROWS)

    # data-driven quotas (max over cores)
    lc = ec - owner * CPC                          # local clause id
    win1 = lc >> 7
    ch1 = np.searchsorted(b1, el, side="right") - 1
    cnt1 = np.bincount(((owner * NWIN1 + win1) * (len(b1) - 1) + ch1).astype(np.int64),
                       minlength=NCORES * NWIN1 * (len(b1) - 1))
    cnt1 = cnt1.reshape(NCORES, NWIN1, len(b1) - 1)
    q1 = [max(1, int(math.ceil(cnt1[:, :, c].max() / P))) for c in range(len(b1) - 1)]

    win2 = el >> 7
    ch2 = np.searchsorted(b2, lc, side="right") - 1
    cnt2 = np.bincount(((owner * NWIN2 + win2) * (len(b2) - 1) + ch2).astype(np.int64),
                       minlength=NCORES * NWIN2 * (len(b2) - 1))
    cnt2 = cnt2.reshape(NCORES, NWIN2, len(b2) - 1)
    q2 = [max(1, int(math.ceil(cnt2[:, :, c].max() / P))) for c in range(len(b2) - 1)]

    lit16 = np.ascontiguousarray(lit_feat.astype(tdt))

    # reduce-scatter split count: chunks overlap the collective with phase 2
    RSC = 1
    for cand in (7, 5, 4, 3, 2):
        if NW3 % cand == 0 and NG2 % cand == 0:
            RSC = cand
            break

    def _cat_groups(streams, ngroup):
        """Concat per-chunk idx streams group-block-wise into one array."""
        widths = [s.shape[1] // ngroup for s in streams]
        out = np.empty((P, ngroup * sum(widths)), streams[0].dtype)
        o = 0
        for g in range(ngroup):
            for s, w in zip(streams, widths):
                out[:, o:o + w] = s[:, g * w:(g + 1) * w]
                o += w
        return out

    def _interleave(a, b, ngroup):
        """Per-group [a_block | b_block] interleave of two [P, ngroup*w] arrays."""
        w = a.shape[1] // ngroup
        out = np.empty((P, ngroup * 2 * w), a.dtype)
        for g in range(ngroup):
            out[:, g * 2 * w:g * 2 * w + w] = a[:, g * w:(g + 1) * w]
            out[:, g * 2 * w + w:(g + 1) * 2 * w] = b[:, g * w:(g + 1) * w]
        return out

    in_maps = []
    meta = None
    for k in range(NCORES):
        m = owner == k
        elk, eck, lck = el[m], ec[m], lc[m]
        idx1, rel1, rcp1, ncols1, cb1 = _build_streams(
            lck, elk, recipc[eck], NWIN1, b1, q1, NG1)
        idx2, rel2, _, ncols2, cb2 = _build_streams(
            elk * 1, lck, None, NWIN2, b2, q2, NG2)

        # aux rows over this core's padded clause rows
        cl_ids = np.arange(CLROWS) + k * CPC
        valid = cl_ids < n_clause
        cl_ids = np.minimum(cl_ids, n_clause - 1)
        a_has = np.where(valid, hasc[cl_ids], 0.0).astype(tdt)[None, :]
        a_cf = np.where(valid, clause_feat[cl_ids, 0], 0.0)
        a_ones = valid.astype(np.float32)
        a_cf2 = np.stack([a_cf, a_ones]).astype(tdt)

        # finalize: per-partition recip over this core's interleaved lit slice
        CH, CHS = TROWS // RSC, SLICE // RSC
        w_all = np.arange(NW3)
        c_of_w = w_all // (NW3 // RSC)
        loc_of_w = w_all % (NW3 // RSC)
        base = c_of_w * CH + k * CHS + loc_of_w * P
        lit_ids = base[:, None] + np.arange(P)[None, :]     # [NW3, P]
        lvalid = lit_ids < n_lit
        lit_ids = np.minimum(lit_ids, n_lit - 1)
        rlit = np.where(lvalid, 1.0 / np.maximum(degl[lit_ids], 1.0), 1.0)
        rlit = rlit.astype(np.float32).T.copy()             # [128, NW3]

        iota_sb = np.broadcast_to(np.arange(P, dtype=tdt), (P, P)).copy()

        im = {
            "lit16": lit16,
            "idxc1": _cat_groups(idx1, NG1),
            "idxc2": _cat_groups(idx2, NG2),
            "relrcp1": _interleave(rel1, rcp1, NG1),
            "rel2": rel2,
            "auxhas": a_has, "auxcf2": a_cf2,
            "rlit": rlit, "iota": iota_sb,
            "wl2c": W_l2c.astype(tdt),
            "brow": b_l2c.astype(tdt)[None, :],
            "wc2l": W_c2l[:D].astype(tdt),
            "wb2": np.stack([W_c2l[D], b_c2l]).astype(tdt),
        }
        in_maps.append(im)
        if meta is None:
            meta = dict(
                n_lit=n_lit, n_clause=n_clause, CPC=CPC,
                NWIN1=NWIN1, NG1=NG1, CLROWS=CLROWS,
                NWIN2=NWIN2, NG2=NG2, TROWS=TROWS, SLICE=SLICE, NW3=NW3,
                b1=b1, b2=b2, q1=q1, q2=q2, RSC=RSC,
                ncols1=ncols1, cb1=cb1, ncols2=ncols2, cb2=cb2,
            )
    return meta, in_maps


# ----------------------------------------------------------------------------
# bass program
# ----------------------------------------------------------------------------

def _build_nc(meta, reps=1, skip_rs=False):
    import concourse.bass as bass
    import concourse.bacc as bacc
    import concourse.mybir as mybir
    import concourse.tile as tile

    tdt = mybir.dt.float16 if F16 else mybir.dt.float32
    f32 = mybir.dt.float32

    NG1, NWIN1, ncols1, cb1, q1 = meta["NG1"], meta["NWIN1"], meta["ncols1"], meta["cb1"], meta["q1"]
    NG2, NWIN2, ncols2, cb2, q2 = meta["NG2"], meta["NWIN2"], meta["ncols2"], meta["cb2"], meta["q2"]
    CLROWS, TROWS, SLICE, NW3 = meta["CLROWS"], meta["TROWS"], meta["SLICE"], meta["NW3"]
    RSC = meta["RSC"]
    b1, b2 = meta["b1"], meta["b2"]
    nch1, nch2 = len(b1) - 1, len(b2) - 1
    n_lit = meta["n_lit"]
    CW1, CW2 = ncols1 * 8, ncols2 * 8          # idx cols (int16) per group
    WPC = NW3 // RSC                           # finalize windows per RS chunk
    GPC2 = NG2 // RSC                          # phase-2 groups per RS chunk

    nc = bacc.Bacc("TRN2", target_bir_lowering=False, debug=False,
                   num_devices=NCORES, num_swdge_queues=NQ)

    lit16 = nc.declare_dram_parameter("lit16", [n_lit, D], tdt, isOutput=False)
    idxc1 = nc.declare_dram_parameter("idxc1", [P, NG1 * CW1], mybir.dt.int16, isOutput=False)
    idxc2 = nc.declare_dram_parameter("idxc2", [P, NG2 * CW2], mybir.dt.int16, isOutput=False)
    relrcp1 = nc.declare_dram_parameter("relrcp1", [P, NG1 * 2 * ncols1], tdt, isOutput=False)
    rel2 = nc.declare_dram_parameter("rel2", [P, NG2 * ncols2], tdt, isOutput=False)
    auxhas = nc.declare_dram_parameter("auxhas", [1, CLROWS], tdt, isOutput=False)
    auxcf2 = nc.declare_dram_parameter("auxcf2", [2, CLROWS], tdt, isOutput=False)
    rlit = nc.declare_dram_parameter("rlit", [P, NW3], f32, isOutput=False)
    iota_e = nc.declare_dram_parameter("iota", [P, P], tdt, isOutput=False)
    wl2c_e = nc.declare_dram_parameter("wl2c", [D, D], tdt, isOutput=False)
    brow_e = nc.declare_dram_parameter("brow", [1, D], tdt, isOutput=False)
    wc2l_e = nc.declare_dram_parameter("wc2l", [D, D], tdt, isOutput=False)
    wb2_e = nc.declare_dram_parameter("wb2", [2, D], tdt, isOutput=False)
    out_e = nc.declare_dram_parameter("out", [SLICE, D], f32, isOutput=True)

    wh_tbl = nc.dram_tensor("wh_tbl", [CLROWS, D], tdt)
    t_tbl = nc.dram_tensor("t_tbl", [TROWS, D], tdt)
    t_red = nc.dram_tensor("t_red", [SLICE, D], tdt)

    # Tile round-robins Pool DMAs over 8 DMASW sem lanes in emission order;
    # aligning queue_num with that rotation keeps each sem lane single-queue
    # (required: a DMA sem is locked to one SWDGE queue).
    pool_dma_count = [0]

    def _next_q():
        q = pool_dma_count[0] % NQ
        pool_dma_count[0] += 1
        return q

    with tile.TileContext(nc) as tc:
        with (
            tc.tile_pool(name="const", bufs=1) as cpool,
            tc.tile_pool(name="stage", bufs=3) as stage,
            tc.tile_pool(name="memb", bufs=3) as membp,
            tc.tile_pool(name="aux", bufs=3) as auxp,
            tc.tile_pool(name="small", bufs=4) as small,
            tc.tile_pool(name="psum", bufs=2, space="PSUM") as psum,
        ):
            iota_t = cpool.tile([P, P], tdt, tag="iota")
            nc.sync.dma_start(out=iota_t[:], in_=iota_e[:, :])
            wl2c_t = cpool.tile([D, D], tdt, tag="wl2c")
            nc.sync.dma_start(out=wl2c_t[:], in_=wl2c_e[:, :])
            brow_t = cpool.tile([1, D], tdt, tag="brow")
            nc.sync.dma_start(out=brow_t[:], in_=brow_e[:, :])
            wc2l_t = cpool.tile([D, D], tdt, tag="wc2l")
            nc.sync.dma_start(out=wc2l_t[:], in_=wc2l_e[:, :])
            wb2_t = cpool.tile([2, D], tdt, tag="wb2")
            nc.sync.dma_start(out=wb2_t[:], in_=wb2_e[:, :])
            rlit_t = cpool.tile([P, NW3], f32, tag="rlit")
            nc.sync.dma_start(out=rlit_t[:], in_=rlit[:, :])

            for rep in range(reps):
                # ---------------- phase 1 ----------------
                for g in range(NG1):
                    st = stage.tile([P, ncols1, D], tdt, tag="st1")
                    it = small.tile([P, CW1], mybir.dt.int16, tag="i1")
                    nc.sync.dma_start(out=it[:], in_=idxc1[:, g * CW1:(g + 1) * CW1])
                    for c in range(nch1):
                        tot = WG * q1[c]          # staging columns for this chunk
                        # HW desc-ring limit: <=1024 idx per dma_gather call
                        for o in range(0, tot, 8):
                            n = min(8, tot - o)
                            nc.gpsimd.dma_gather(
                                out_ap=st[:, cb1[c] + o:cb1[c] + o + n, :],
                                in_ap=lit16[b1[c]:b1[c + 1], :],
                                idxs_ap=it[:, (cb1[c] + o) * 8:(cb1[c] + o + n) * 8],
                                num_idxs=n * P,
                                num_idxs_reg=n * P,
                                elem_size=D,
                                queue_num=_next_q(),
                            )
                    rr = auxp.tile([P, 2 * ncols1], tdt, tag="rr1")
                    nc.sync.dma_start(
                        out=rr[:], in_=relrcp1[:, g * 2 * ncols1:(g + 1) * 2 * ncols1])
                    rl = rr[:, 0:ncols1]
                    rc = rr[:, ncols1:2 * ncols1]
                    mb = membp.tile([P, ncols1, P], tdt, tag="mb1")
                    nc.vector.tensor_tensor(
                        out=mb[:],
                        in0=iota_t[:, None, :].to_broadcast([P, ncols1, P]),
                        in1=rl.to_broadcast([P, ncols1, P]),
                        op=mybir.AluOpType.is_equal,
                    )
                    # fold 1/deg(clause) into the gathered messages (per slot)
                    nc.vector.tensor_tensor(
                        out=st[:],
                        in0=st[:],
                        in1=rc[:, :, None].to_broadcast([P, ncols1, D]),
                        op=mybir.AluOpType.mult,
                    )
                    ahas = auxp.tile([1, WG * P], tdt, tag="ahas")
                    nc.sync.dma_start(out=ahas[:], in_=auxhas[:, g * WG * P:(g + 1) * WG * P])
                    acf2 = auxp.tile([2, WG * P], tdt, tag="acf2")
                    nc.sync.dma_start(out=acf2[:], in_=auxcf2[:, g * WG * P:(g + 1) * WG * P])

                    whg = small.tile([P, WG, P], tdt, tag="whg")
                    for w in range(WG):
                        acc = psum.tile([P, P], f32, space="PSUM", tag="acc1")
                        cols = []
                        for c in range(nch1):
                            cols += [cb1[c] + w * q1[c] + t for t in range(q1[c])]
                        for i, col in enumerate(cols):
                            nc.tensor.matmul(
                                out=acc[:], lhsT=st[:, col, :], rhs=mb[:, col, :],
                                start=(i == 0), stop=(i == len(cols) - 1),
                            )
                        # acc already holds meanT (1/deg folded at gather)
                        meanT = small.tile([P, P], tdt, tag="meanT")
                        nc.vector.tensor_copy(out=meanT[:], in_=acc[:])
                        # cembsT = relu(W^T @ meanT + b x has)
                        p2t = psum.tile([P, P], f32, space="PSUM", tag="proj1")
                        nc.tensor.matmul(out=p2t[:], lhsT=wl2c_t[:], rhs=meanT[:],
                                         start=True, stop=False)
                        nc.tensor.matmul(out=p2t[:], lhsT=brow_t[:],
                                         rhs=ahas[:, w * P:(w + 1) * P],
                                         start=False, stop=True)
                        cembsT = small.tile([P, P], tdt, tag="cembsT")
                        nc.scalar.activation(out=cembsT[:], in_=p2t[:],
                                             func=mybir.ActivationFunctionType.Relu)
                        # wh = cembs @ Wc2l + cf x W[128] + 1 x b
                        p3t = psum.tile([P, P], f32, space="PSUM", tag="proj2")
                        nc.tensor.matmul(out=p3t[:], lhsT=cembsT[:], rhs=wc2l_t[:],
                                         start=True, stop=False)
                        nc.tensor.matmul(out=p3t[:], lhsT=acf2[:, w * P:(w + 1) * P],
                                         rhs=wb2_t[:], start=False, stop=True)
                        nc.scalar.copy(out=whg[:, w, :], in_=p3t[:])
                    row0 = g * WG * P
                    nc.scalar.dma_start(
                        out=wh_tbl[row0:row0 + WG * P, :].rearrange(
                            "(w p) f -> p w f", p=P),
                        in_=whg[:])

                # ---------------- phase 2 (+ interleaved RS/finalize) --------
                def rs_start(cidx):
                    CH, CHS = TROWS // RSC, SLICE // RSC
                    if not skip_rs:
                        nc.gpsimd.collective_compute(
                            "ReduceScatter",
                            mybir.AluOpType.add,
                            replica_groups=[list(range(NCORES))],
                            ins=[t_tbl[cidx * CH:(cidx + 1) * CH, :]],
                            outs=[t_red[cidx * CHS:(cidx + 1) * CHS, :]],
                        )

                def fin_chunk(cidx):
                    # scalar-engine DMAs: keeps the sync queue (group idx/rel
                    # loads) from stalling behind RS completion
                    CH, CHS = TROWS // RSC, SLICE // RSC
                    src_fin = t_tbl if skip_rs else t_red
                    fin = small.tile([P, WPC, P], tdt, tag="fin_in")
                    nc.scalar.dma_start(
                        out=fin[:],
                        in_=src_fin[cidx * CHS:(cidx + 1) * CHS, :].rearrange(
                            "(w p) f -> p w f", p=P))
                    og = small.tile([P, WPC, P], f32, tag="fin_out")
                    for w2 in range(WPC):
                        wabs = cidx * WPC + w2
                        nc.scalar.activation(out=og[:, w2, :], in_=fin[:, w2, :],
                                             func=mybir.ActivationFunctionType.Relu,
                                             scale=rlit_t[:, wabs:wabs + 1])
                    nc.scalar.dma_start(
                        out=out_e[cidx * CHS:(cidx + 1) * CHS, :].rearrange(
                            "(w p) f -> p w f", p=P),
                        in_=og[:])

                for g in range(NG2):
                    st = stage.tile([P, ncols2, D], tdt, tag="st2")
                    it = small.tile([P, CW2], mybir.dt.int16, tag="i2")
                    nc.sync.dma_start(out=it[:], in_=idxc2[:, g * CW2:(g + 1) * CW2])
                    for c in range(nch2):
                        tot = WG * q2[c]
                        for o in range(0, tot, 8):
                            n = min(8, tot - o)
                            nc.gpsimd.dma_gather(
                                out_ap=st[:, cb2[c] + o:cb2[c] + o + n, :],
                                in_ap=wh_tbl[b2[c]:b2[c + 1], :],
                                idxs_ap=it[:, (cb2[c] + o) * 8:(cb2[c] + o + n) * 8],
                                num_idxs=n * P,
                                num_idxs_reg=n * P,
                                elem_size=D,
                                queue_num=_next_q(),
                            )
                    rl = auxp.tile([P, ncols2], tdt, tag="rl2")
                    nc.sync.dma_start(out=rl[:], in_=rel2[:, g * ncols2:(g + 1) * ncols2])
                    mb = membp.tile([P, ncols2, P], tdt, tag="mb2")
                    nc.vector.tensor_tensor(
                        out=mb[:],
                        in0=iota_t[:, None, :].to_broadcast([P, ncols2, P]),
                        in1=rl.to_broadcast([P, ncols2, P]),
                        op=mybir.AluOpType.is_equal,
                    )
                    tg = small.tile([P, WG, P], tdt, tag="tg")
                    for w in range(WG):
                        acc = psum.tile([P, P], f32, space="PSUM", tag="acc2")
                        cols = []
                        for c in range(nch2):
                            cols += [cb2[c] + w * q2[c] + t for t in range(q2[c])]
                        for i, col in enumerate(cols):
                            nc.tensor.matmul(
                                out=acc[:], lhsT=mb[:, col, :], rhs=st[:, col, :],
                                start=(i == 0), stop=(i == len(cols) - 1),
                            )
                        nc.scalar.copy(out=tg[:, w, :], in_=acc[:])
                    row0 = g * WG * P
                    nc.scalar.dma_start(
                        out=t_tbl[row0:row0 + WG * P, :].rearrange(
                            "(w p) f -> p w f", p=P),
                        in_=tg[:])
                    if NG2 % RSC == 0 and (g + 1) % GPC2 == 0:
                        cidx = (g + 1) // GPC2 - 1
                        rs_start(cidx)
                        if cidx >= 1:
                            fin_chunk(cidx - 1)   # one chunk behind its RS
                if NG2 % RSC == 0:
                    fin_chunk(RSC - 1)
                else:
                    for cidx in range(RSC):
                        rs_start(cidx)
                        fin_chunk(cidx)

    nc.compile()
    return nc


# ----------------------------------------------------------------------------
# SPMD runner (jitted shard_map over the 8 NeuronCores, cached for reuse)
# ----------------------------------------------------------------------------

class SpmdRunner:
    def __init__(self, nc, n_cores):
        import jax
        import concourse.mybir as mybir
        from concourse.bass2jax import (
            _bass_exec_p, install_neuronx_cc_hook, partition_id_tensor)
        from jax.sharding import Mesh, PartitionSpec
        from jax.experimental.shard_map import shard_map

        install_neuronx_cc_hook()
        self.jax = jax
        self.n_cores = n_cores
        partition_name = nc.partition_id_tensor.name if nc.partition_id_tensor else None
        in_names, out_names, out_avals, zero_shapes = [], [], [], []
        for alloc in nc.m.functions[0].allocations:
            if not isinstance(alloc, mybir.MemoryLocationSet):
                continue
            name = alloc.memorylocations[0].name
            if alloc.kind == "ExternalInput":
                if name != partition_name:
                    in_names.append(name)
            elif alloc.kind == "ExternalOutput":
                out_names.append(name)
                shape = tuple(alloc.tensor_shape)
                dtype = mybir.dt.np(alloc.dtype)
                out_avals.append(jax.core.ShapedArray(shape, dtype))
                zero_shapes.append((shape, dtype))
        self.in_names, self.out_names = in_names, out_names
        self.out_avals, self.zero_shapes = out_avals, zero_shapes
        n_params, n_outs = len(in_names), len(out_avals)
        all_in_names = list(in_names) + list(out_names)
        if partition_name is not None:
            all_in_names.append(partition_name)

        def _body(*args):
            operands = list(args)
            if partition_name is not None:
                operands.append(partition_id_tensor())
            outs = _bass_exec_p.bind(
                *operands,
                out_avals=tuple(out_avals),
                in_names=tuple(all_in_names),
                out_names=tuple(out_names),
                lowering_input_output_aliases=(),
                sim_require_finite=True,
                sim_require_nnan=True,
                nc=nc,
            )
            return tuple(outs)

        devices = jax.devices()[:n_cores]
        self.mesh = Mesh(np.asarray(devices), ("core",))
        in_specs = (PartitionSpec("core"),) * (n_params + n_outs)
        out_specs = (PartitionSpec("core"),) * n_outs
        self.fn = jax.jit(
            shard_map(_body, mesh=self.mesh, in_specs=in_specs,
                      out_specs=out_specs, check_rep=False),
            keep_unused=True,
        )
        self._device_args = None
        self._pspec = PartitionSpec

    def put_inputs(self, in_maps):
        jax = self.jax
        n = self.n_cores
        sharding = jax.sharding.NamedSharding(self.mesh, self._pspec("core"))
        args = []
        for name in self.in_names:
            cat = np.concatenate([np.asarray(in_maps[c][name]) for c in range(n)], axis=0)
            args.append(jax.device_put(cat, sharding))
        for shape, dtype in self.zero_shapes:
            z = np.zeros((n * shape[0], *shape[1:]), dtype)
            args.append(jax.device_put(z, sharding))
        self._device_args = args
        jax.block_until_ready(args)

    def run(self):
        outs = self.fn(*self._device_args)
        self.jax.block_until_ready(outs)
        return outs

    def results(self, outs):
        n = self.n_cores
        res = []
        for c in range(n):
            d = {}
            for i, name in enumerate(self.out_names):
                shp = self.out_avals[i].shape
                d[name] = np.asarray(outs[i]).reshape(n, *shp)[c]
            res.append(d)
        return res


# ----------------------------------------------------------------------------
# public entry point
# ----------------------------------------------------------------------------

_CACHE = {}


def _get_runner(meta, reps):
    key = (tuple(sorted(meta.items(), key=lambda kv: repr(kv[0]))).__repr__(), reps)
    if key not in _CACHE:
        nc = _build_nc(meta, reps=reps)
        _CACHE[key] = SpmdRunner(nc, NCORES)
    return _CACHE[key]


def assemble(meta, res):
    """Reassemble per-core RS-chunked output slices into the full table."""
    RSC, TROWS, SLICE = meta["RSC"], meta["TROWS"], meta["SLICE"]
    CH, CHS = TROWS // RSC, SLICE // RSC
    full = np.empty((TROWS, D), np.float32)
    for c in range(RSC):
        for k in range(NCORES):
            full[c * CH + k * CHS:c * CH + (k + 1) * CHS] = \
                res[k]["out"][c * CHS:(c + 1) * CHS]
    return np.ascontiguousarray(full[:meta["n_lit"]])


def kernel(**inputs):
    meta, in_maps = _prep(inputs)
    r = _get_runner(meta, reps=1)
    r.put_inputs(in_maps)
    outs = r.run()
    res = r.results(outs)
    return assemble(meta, res)



# revision 2
# speedup vs baseline: 1.0568x; 1.0568x over previous
"""CNF GNN message-passing layer on 8 Trainium2 NeuronCores (Bass/Tile) — v2.

Architecture (edge parallel, clause-owner sharding, one-hot matmul seg-sum):
  - Core k owns clauses [k*CPC, (k+1)*CPC) and both message directions of
    their edges.
  - Windows are 256 rows (vs 128): seg-sum matmul rhs is the [slots, 256]
    one-hot, output accumulates in a [128, 256] fp32 PSUM tile.  Per-group
    quotas (max over the 8 cores only) cut gather padding to ~16-26%.
  - Phase 1 (l2c): gather raw lit rows per edge, seg-sum per clause window,
    then q = relu(sumT@W + deg*b); wh = rcp * (q@W2 + deg1*cf*W2[128] +
    deg1*valid*b2) — exact because rcp*deg1 == 1; the rcp scale rides the
    PSUM->SBUF copy (scalar activation Copy w/ per-partition scale).
  - The lit tail (rows >= 98304, beyond 3 int16 chunks) uses one group-wide
    column with a 1024-wide one-hot (4 matmul slices).
  - Phase 2 (c2l): gather wh rows per edge, seg-sum into lit windows ->
    partial table T_k, ReduceScatter(add) in RSC chunks interleaved with
    phase-2 groups, finalize mean (1/deg) + relu.
  - Per-group streams (gather idx + one-hot rel ids + rcp) are packed into a
    single int16 blob -> one sync DMA per group (+1 tiny aux DMA in p1).
"""
import sys
sys.path.insert(0, "/opt/trn_rl_repo")

import math
import numpy as np

P = 128
D = 128
NCORES = 8
W = 256            # window rows
WPG = 4            # windows per group
GR = W * WPG       # group rows (1024)
CH = 32768         # int16 chunk size
NQ = 4             # SWDGE queues
CALL_COLS = 8      # staging cols per dma_gather call (1024 idx HW cap)


def _ceil_to(x, m):
    return (x + m - 1) // m * m


def _wrap(idx16):
    """Flat slot-ordered idx array -> [128, n/16] wrapped+replicated layout."""
    n = idx16.size
    arr = idx16.reshape(n // 16, 16).T.reshape(16, -1)
    return np.tile(arr, (8, 1))


def _prep(inputs):
    lit_feat = np.asarray(inputs["lit_feat"], np.float32)
    clause_feat = np.asarray(inputs["clause_feat"], np.float32)
    el = np.asarray(inputs["edge_lit"]).astype(np.int64)
    ec = np.asarray(inputs["edge_clause"]).astype(np.int64)
    W_l2c = np.asarray(inputs["W_l2c"], np.float32)
    b_l2c = np.asarray(inputs["b_l2c"], np.float32)
    W_c2l = np.asarray(inputs["W_c2l"], np.float32)
    b_c2l = np.asarray(inputs["b_c2l"], np.float32)

    n_lit = lit_feat.shape[0]
    n_clause = clause_feat.shape[0]
    tdt = np.float16

    CPC = n_clause // NCORES
    CLROWS = _ceil_to(CPC, GR)
    NG1 = CLROWS // GR
    TROWS = _ceil_to(n_lit, GR)
    NG2 = TROWS // GR
    SLICE = TROWS // NCORES
    NW3 = SLICE // P
    RSC = 1
    for cand in (7, 5, 4, 3, 2):
        if NW3 % cand == 0 and NG2 % cand == 0:
            RSC = cand
            break
    GPC2 = NG2 // RSC

    # phase-1 main chunks over lit rows + per-group tail column
    NCH1 = n_lit // CH                      # full chunks (3)
    TAIL0 = NCH1 * CH                       # 98304
    # phase-2 chunks over CLROWS
    b2 = list(range(0, CLROWS, CH)) + [CLROWS]
    NCH2 = len(b2) - 1

    degc = np.bincount(ec, minlength=n_clause).astype(np.float32)
    degl = np.bincount(el, minlength=n_lit).astype(np.float32)
    recipc = (1.0 / np.maximum(degc, 1.0)).astype(np.float32)

    owner = ec // CPC
    lc = ec - owner * CPC

    # ---- quotas (shared across cores: max over cores) ----
    g1 = lc // GR
    wl1 = (lc // W) % WPG
    ch1 = np.minimum(el // CH, NCH1)
    cnt1 = np.zeros((NCORES, NG1, WPG, NCH1 + 1), np.int64)
    np.add.at(cnt1, (owner, g1, wl1, ch1), 1)
    q1 = np.ceil(cnt1[:, :, :, :NCH1].max(axis=0) / P).astype(int)   # [NG1,WPG,NCH1]
    q1t = np.ceil(cnt1[:, :, :, NCH1].sum(axis=2).max(axis=0) / P).astype(int)  # [NG1]

    g2 = el // GR
    wl2 = (el // W) % WPG
    ch2 = lc // CH
    cnt2 = np.zeros((NCORES, NG2, WPG, NCH2), np.int64)
    np.add.at(cnt2, (owner, g2, wl2, ch2), 1)
    q2 = np.ceil(cnt2.max(axis=0) / P).astype(int)                   # [NG2,WPG,NCH2]

    # staging column layout per group: chunk-major for contiguous gather calls
    # p1: [c0 cols (w0..w3) | c1 | c2 | tail]; mb built over main cols only.
    def _layout1(g):
        off = {}
        col = 0
        for c in range(NCH1):
            for w in range(WPG):
                off[(w, c)] = col
                col += q1[g, w, c]
        moff = col                      # tail cols start here
        return off, moff, moff + q1t[g]

    def _layout2(g):
        off = {}
        col = 0
        for c in range(NCH2):
            for w in range(WPG):
                off[(w, c)] = col
                col += q2[g, w, c]
        return off, col

    sec1 = []    # per group: (colcnt per chunk, ncols main, ncols total, blob width)
    for g in range(NG1):
        ccols = [int(q1[g, :, c].sum()) for c in range(NCH1)] + [int(q1t[g])]
        nm = sum(ccols[:NCH1])
        nt = nm + ccols[NCH1]
        w_blob = nt * 8 + nm + ccols[NCH1] + 8   # idx + rel(main) + relt + rcp
        sec1.append((ccols, nm, nt, w_blob))
    sec2 = []
    for g in range(NG2):
        ccols = [int(q2[g, :, c].sum()) for c in range(NCH2)]
        nm = sum(ccols)
        w_blob = nm * 8 + nm
        sec2.append((ccols, nm, nm, w_blob))

    MC1 = max(s[2] for s in sec1)       # max total staging cols p1
    MM1 = max(s[1] for s in sec1)
    MT1 = max(int(x) for x in q1t)
    MC2 = max(s[2] for s in sec2)
    BM1 = max(s[3] for s in sec1)       # max blob width p1
    BM2 = max(s[3] for s in sec2)
    BW1 = sum(s[3] for s in sec1)       # total blob width p1
    BW2 = sum(s[3] for s in sec2)
    boff1 = np.cumsum([0] + [s[3] for s in sec1])
    boff2 = np.cumsum([0] + [s[3] for s in sec2])

    lit16 = np.ascontiguousarray(lit_feat.astype(tdt))

    in_maps = []
    for k in range(NCORES):
        m = owner == k
        elk, lck = el[m], lc[m]

        # ---------------- phase 1 streams ----------------
        blob1 = np.zeros((P, BW1), np.int16)
        kg1 = lck // GR
        kwl = (lck // W) % WPG
        kch = np.minimum(elk // CH, NCH1)
        order = np.lexsort((elk, kch, kwl, kg1))
        e_l, l_c = elk[order], lck[order]
        e_g, e_w, e_c = kg1[order], kwl[order], kch[order]
        # rank within (g, w, c)
        key = ((e_g * WPG + e_w) * (NCH1 + 1)) + e_c
        uniq, starts = np.unique(key, return_index=True)
        run_start = np.zeros(len(key), np.int64)
        run_start[starts] = starts
        run_start = np.maximum.accumulate(run_start)
        rank = np.arange(len(key)) - run_start

        for g in range(NG1):
            ccols, nm, nt, wb = sec1[g]
            off, moff, _ = _layout1(g)
            base = boff1[g]
            idx = np.zeros(nt * P, np.int16)
            rel = np.full((P, nm), -1.0, np.float16)
            relt = np.full((P, ccols[NCH1]), -1.0, np.float16)
            sel = e_g == g
            sl_l, sl_c = e_l[sel], l_c[sel]
            sl_w, sl_ch, sl_r = e_w[sel], e_c[sel], rank[sel]
            mn = sl_ch < NCH1
            # main slots
            colm = np.zeros(len(sl_l), np.int64)
            for w in range(WPG):
                for c in range(NCH1):
                    mm = (sl_w == w) & (sl_ch == c)
                    colm[mm] = off[(w, c)] + (sl_r[mm] >> 7)
            # tail slots: group-wide column(s), rank across the whole group
            tm = ~mn
            trank = np.cumsum(tm) - 1
            colm[tm] = moff + (trank[tm] >> 7)
            p = np.where(mn, sl_r & 127, trank[tm.cumsum() * 0 + 0] * 0)[
                :] if False else np.where(mn, sl_r & 127, 0)
            p[tm] = trank[tm] & 127
            slot = colm * P + p
            idx[slot] = np.where(mn, sl_l - sl_ch * CH, sl_l - TAIL0).astype(np.int16)
            rel[p[mn], colm[mn]] = (sl_c[mn] % W).astype(np.float16)
            relt[p[tm], colm[tm] - moff] = (sl_c[tm] - g * GR).astype(np.float16)

            blob1[:, base:base + nt * 8] = _wrap(idx)
            blob1[:, base + nt * 8:base + nt * 8 + nm] = rel.view(np.int16)
            o2 = base + nt * 8 + nm
            blob1[:, o2:o2 + ccols[NCH1]] = relt.view(np.int16)
            o3 = o2 + ccols[NCH1]
            cid = k * CPC + g * GR + np.arange(GR)
            cid = np.minimum(cid, n_clause - 1)
            rcp = recipc[cid].astype(np.float16).reshape(8, P).T  # [P, 8]
            blob1[:, o3:o3 + 8] = rcp.view(np.int16)

        rcpc = np.zeros((P, NG1 * 8), np.float32)
        for g in range(NG1):
            cid = k * CPC + g * GR + np.arange(GR)
            cid = np.minimum(cid, n_clause - 1)
            rcpc[:, g * 8:(g + 1) * 8] = recipc[cid].reshape(8, P).T

        # phase-1 aux rows [3, CLROWS]: deg, cf*deg1, valid*deg1
        cid = k * CPC + np.arange(CLROWS)
        valid = cid < n_clause
        cid = np.minimum(cid, n_clause - 1)
        dg = np.where(valid, degc[cid], 0.0)
        dg1 = np.maximum(dg, 1.0)
        degrow = dg[None, :].astype(np.float16)
        acf2 = np.stack([
            np.where(valid, clause_feat[cid, 0], 0.0) * dg1,
            valid.astype(np.float32) * dg1,
        ]).astype(np.float16)

        # ---------------- phase 2 streams ----------------
        blob2 = np.zeros((P, BW2), np.int16)
        kg2 = elk // GR
        kw2 = (elk // W) % WPG
        kc2 = lck // CH
        order2 = np.lexsort((lck, kc2, kw2, kg2))
        e2_l, l2_c = elk[order2], lck[order2]
        e2_g, e2_w, e2_c = kg2[order2], kw2[order2], kc2[order2]
        key2 = (e2_g * WPG + e2_w) * NCH2 + e2_c
        uniq2, starts2 = np.unique(key2, return_index=True)
        run2 = np.zeros(len(key2), np.int64)
        run2[starts2] = starts2
        run2 = np.maximum.accumulate(run2)
        rank2 = np.arange(len(key2)) - run2

        for g in range(NG2):
            ccols, nm, nt, wb = sec2[g]
            off, _ = _layout2(g)
            base = boff2[g]
            idx = np.zeros(nm * P, np.int16)
            rel = np.full((P, nm), -1.0, np.float16)
            sel = e2_g == g
            s_l, s_c = e2_l[sel], l2_c[sel]
            s_w, s_ch, s_r = e2_w[sel], e2_c[sel], rank2[sel]
            colm = np.zeros(len(s_l), np.int64)
            for w in range(WPG):
                for c in range(NCH2):
                    mm = (s_w == w) & (s_ch == c)
                    colm[mm] = off[(w, c)] + (s_r[mm] >> 7)
            p = s_r & 127
            slot = colm * P + p
            idx[slot] = (s_c - s_ch * CH).astype(np.int16)
            rel[p, colm] = (s_l % W).astype(np.float16)
            blob2[:, base:base + nm * 8] = _wrap(idx)
            blob2[:, base + nm * 8:base + nm * 8 + nm] = rel.view(np.int16)

        # finalize reciprocals, interleaved RS-chunk layout (as baseline)
        CHR, CHS = TROWS // RSC, SLICE // RSC
        w_all = np.arange(NW3)
        c_of_w = w_all // (NW3 // RSC)
        loc_of_w = w_all % (NW3 // RSC)
        rbase = c_of_w * CHR + k * CHS + loc_of_w * P
        lit_ids = rbase[:, None] + np.arange(P)[None, :]
        lvalid = lit_ids < n_lit
        lit_ids = np.minimum(lit_ids, n_lit - 1)
        rlit = np.where(lvalid, 1.0 / np.maximum(degl[lit_ids], 1.0), 1.0)
        rlit = rlit.astype(np.float32).T.copy()

        iota256 = np.broadcast_to(np.arange(W, dtype=tdt), (P, W)).copy()
        iota1k = np.broadcast_to(np.arange(GR, dtype=tdt), (P, GR)).copy()

        im = {
            "lit16": lit16,
            "blob1": blob1, "blob2": blob2,
            "degrow": degrow, "acf2": acf2, "rcpc": rcpc,
            "rlit": rlit, "iota256": iota256, "iota1k": iota1k,
            "wl2c": W_l2c.astype(tdt),
            "brow": b_l2c.astype(tdt)[None, :],
            "wc2l": W_c2l[:D].astype(tdt),
            "wb2": np.stack([W_c2l[D], b_c2l]).astype(tdt),
        }
        in_maps.append(im)

    meta = dict(
        n_lit=n_lit, n_clause=n_clause, CPC=CPC,
        NG1=NG1, CLROWS=CLROWS, NG2=NG2, TROWS=TROWS,
        SLICE=SLICE, NW3=NW3, RSC=RSC, GPC2=GPC2,
        NCH1=NCH1, TAIL0=TAIL0, NCH2=NCH2, b2=tuple(b2),
        q1=tuple(map(tuple, map(tuple, q1.reshape(NG1, -1)))),
        q1t=tuple(int(x) for x in q1t),
        q2=tuple(map(tuple, map(tuple, q2.reshape(NG2, -1)))),
        sec1=tuple((tuple(s[0]), s[1], s[2], s[3]) for s in sec1),
        sec2=tuple((tuple(s[0]), s[1], s[2], s[3]) for s in sec2),
        boff1=tuple(int(x) for x in boff1), boff2=tuple(int(x) for x in boff2),
        MC1=MC1, MM1=MM1, MT1=MT1, MC2=MC2, BW1=BW1, BW2=BW2,
        BM1=BM1, BM2=BM2,
    )
    return meta, in_maps


# ----------------------------------------------------------------------------
# bass program
# ----------------------------------------------------------------------------

def _build_nc(meta, reps=1):
    import concourse.bass as bass
    import concourse.bacc as bacc
    import concourse.mybir as mybir
    import concourse.tile as tile

    tdt = mybir.dt.float16
    f32 = mybir.dt.float32
    i16 = mybir.dt.int16

    n_lit = meta["n_lit"]
    NG1, NG2 = meta["NG1"], meta["NG2"]
    CLROWS, TROWS, SLICE, NW3 = meta["CLROWS"], meta["TROWS"], meta["SLICE"], meta["NW3"]
    RSC, GPC2 = meta["RSC"], meta["GPC2"]
    NCH1, TAIL0, NCH2 = meta["NCH1"], meta["TAIL0"], meta["NCH2"]
    b2 = meta["b2"]
    q1 = [np.array(q).reshape(WPG, NCH1) for q in meta["q1"]]
    q1t = meta["q1t"]
    q2 = [np.array(q).reshape(WPG, NCH2) for q in meta["q2"]]
    sec1, sec2 = meta["sec1"], meta["sec2"]
    boff1, boff2 = meta["boff1"], meta["boff2"]
    MC1, MM1, MT1, MC2 = meta["MC1"], meta["MM1"], meta["MT1"], meta["MC2"]
    BW1, BW2 = meta["BW1"], meta["BW2"]
    BM1, BM2 = meta["BM1"], meta["BM2"]
    WPC = NW3 // RSC

    nc = bacc.Bacc("TRN2", target_bir_lowering=False, debug=False,
                   num_devices=NCORES, num_swdge_queues=NQ)

    lit16 = nc.declare_dram_parameter("lit16", [n_lit, D], tdt, isOutput=False)
    blob1_e = nc.declare_dram_parameter("blob1", [P, BW1], i16, isOutput=False)
    blob2_e = nc.declare_dram_parameter("blob2", [P, BW2], i16, isOutput=False)
    degrow_e = nc.declare_dram_parameter("degrow", [1, CLROWS], tdt, isOutput=False)
    acf2_e = nc.declare_dram_parameter("acf2", [2, CLROWS], tdt, isOutput=False)
    rlit_e = nc.declare_dram_parameter("rlit", [P, NW3], f32, isOutput=False)
    rcpc_e = nc.declare_dram_parameter("rcpc", [P, NG1 * 8], f32, isOutput=False)
    iota256_e = nc.declare_dram_parameter("iota256", [P, W], tdt, isOutput=False)
    iota1k_e = nc.declare_dram_parameter("iota1k", [P, GR], tdt, isOutput=False)
    wl2c_e = nc.declare_dram_parameter("wl2c", [D, D], tdt, isOutput=False)
    brow_e = nc.declare_dram_parameter("brow", [1, D], tdt, isOutput=False)
    wc2l_e = nc.declare_dram_parameter("wc2l", [D, D], tdt, isOutput=False)
    wb2_e = nc.declare_dram_parameter("wb2", [2, D], tdt, isOutput=False)
    out_e = nc.declare_dram_parameter("out", [SLICE, D], f32, isOutput=True)

    wh_tbl = nc.dram_tensor("wh_tbl", [CLROWS, D], tdt)
    t_tbl = nc.dram_tensor("t_tbl", [TROWS, D], tdt)
    t_red = nc.dram_tensor("t_red", [SLICE, D], tdt)

    pool_dma_count = [0]

    def _next_q():
        q = pool_dma_count[0] % NQ
        pool_dma_count[0] += 1
        return q

    with tile.TileContext(nc) as tc:
        with (
            tc.tile_pool(name="const", bufs=1) as cpool,
            tc.tile_pool(name="stage", bufs=3) as stage,
            tc.tile_pool(name="memb", bufs=2) as membp,
            tc.tile_pool(name="blob", bufs=3) as blobp,
            tc.tile_pool(name="aux", bufs=2) as auxp,
            tc.tile_pool(name="small", bufs=3) as small,
            tc.tile_pool(name="fin", bufs=2) as finp,
            tc.tile_pool(name="psA", bufs=2, space="PSUM") as psA,
            tc.tile_pool(name="psB", bufs=2, space="PSUM") as psB,
        ):
            iota256_t = cpool.tile([P, W], tdt, tag="iota256")
            nc.sync.dma_start(out=iota256_t[:], in_=iota256_e[:, :])
            iota1k_t = cpool.tile([P, GR], tdt, tag="iota1k")
            nc.sync.dma_start(out=iota1k_t[:], in_=iota1k_e[:, :])
            wl2c_t = cpool.tile([D, D], tdt, tag="wl2c")
            nc.sync.dma_start(out=wl2c_t[:], in_=wl2c_e[:, :])
            brow_t = cpool.tile([1, D], tdt, tag="brow")
            nc.sync.dma_start(out=brow_t[:], in_=brow_e[:, :])
            wc2l_t = cpool.tile([D, D], tdt, tag="wc2l")
            nc.sync.dma_start(out=wc2l_t[:], in_=wc2l_e[:, :])
            wb2_t = cpool.tile([2, D], tdt, tag="wb2")
            nc.sync.dma_start(out=wb2_t[:], in_=wb2_e[:, :])
            rlit_t = cpool.tile([P, NW3], f32, tag="rlit")
            nc.sync.dma_start(out=rlit_t[:], in_=rlit_e[:, :])
            rcpc_t = cpool.tile([P, NG1 * 8], f32, tag="rcpc")
            nc.sync.dma_start(out=rcpc_t[:], in_=rcpc_e[:, :])

            for rep in range(reps):
                # ---------------- phase 1 ----------------
                for g in range(NG1):
                    ccols, nm, nt, wb = sec1[g]
                    base = boff1[g]
                    bt = blobp.tile([P, BM1], i16, tag="b1")
                    nc.sync.dma_start(out=bt[:, 0:wb], in_=blob1_e[:, base:base + wb])
                    st = stage.tile([P, MC1, D], tdt, tag="st1")
                    col0 = 0
                    for c in range(NCH1 + 1):
                        seccols = ccols[c]
                        if c < NCH1:
                            src = lit16[c * CH:min((c + 1) * CH, n_lit), :]
                        else:
                            src = lit16[TAIL0:n_lit, :]
                        for o in range(0, seccols, CALL_COLS):
                            n = min(CALL_COLS, seccols - o)
                            nc.gpsimd.dma_gather(
                                out_ap=st[:, col0 + o:col0 + o + n, :],
                                in_ap=src,
                                idxs_ap=bt[:, (col0 + o) * 8:(col0 + o + n) * 8],
                                num_idxs=n * P,
                                num_idxs_reg=n * P,
                                elem_size=D,
                                queue_num=_next_q(),
                            )
                        col0 += seccols
                    relv = bt[:, nt * 8:nt * 8 + nm].bitcast(tdt)
                    mb = membp.tile([P, MM1, W], tdt, tag="mb1")
                    nc.vector.tensor_tensor(
                        out=mb[:, 0:nm, :],
                        in0=iota256_t[:, None, :].to_broadcast([P, nm, W]),
                        in1=relv.to_broadcast([P, nm, W]),
                        op=mybir.AluOpType.is_equal,
                    )
                    qt = ccols[NCH1]
                    mbt = membp.tile([P, max(MT1, 1), GR], tdt, tag="mbt1")
                    if qt > 0:
                        reltv = bt[:, nt * 8 + nm:nt * 8 + nm + qt].bitcast(tdt)
                        nc.vector.tensor_tensor(
                            out=mbt[:, 0:qt, :],
                            in0=iota1k_t[:, None, :].to_broadcast([P, qt, GR]),
                            in1=reltv.to_broadcast([P, qt, GR]),
                            op=mybir.AluOpType.is_equal,
                        )
                    dgr = auxp.tile([1, GR], tdt, tag="dgr")
                    nc.sync.dma_start(out=dgr[:], in_=degrow_e[:, g * GR:(g + 1) * GR])
                    ax = auxp.tile([2, GR], tdt, tag="ax1")
                    nc.sync.dma_start(out=ax[:], in_=acf2_e[:, g * GR:(g + 1) * GR])

                    whg = small.tile([P, 2 * WPG, P], tdt, tag="whg")
                    for w in range(WPG):
                        acc = psA.tile([P, W], f32, space="PSUM", tag="acc1")
                        cols = []
                        for c in range(NCH1):
                            o = sum(ccols[:c]) + int(np.sum(q1[g][:w, c]))
                            cols += [o + t for t in range(q1[g][w, c])]
                        tcols = [nm + t for t in range(qt)]
                        ntot = len(cols) + len(tcols)
                        if ntot == 0:
                            nc.vector.memset(whg[:, w * 2, :], 0.0)
                            nc.vector.memset(whg[:, w * 2 + 1, :], 0.0)
                            continue
                        i = 0
                        for col in cols:
                            nc.tensor.matmul(out=acc[:], lhsT=st[:, col, :],
                                             rhs=mb[:, col, :],
                                             start=(i == 0), stop=(i == ntot - 1))
                            i += 1
                        for t in range(qt):
                            nc.tensor.matmul(
                                out=acc[:], lhsT=st[:, nm + t, :],
                                rhs=mbt[:, t, w * W:(w + 1) * W],
                                start=(i == 0), stop=(i == ntot - 1))
                            i += 1
                        accSB = small.tile([P, W], tdt, tag="accSB")
                        nc.vector.tensor_copy(out=accSB[:], in_=acc[:])
                        p2t = psA.tile([P, W], f32, space="PSUM", tag="p2t")
                        nc.tensor.matmul(out=p2t[:], lhsT=wl2c_t[:], rhs=accSB[:],
                                         start=True, stop=False)
                        nc.tensor.matmul(out=p2t[:], lhsT=brow_t[:],
                                         rhs=dgr[0:1, w * W:(w + 1) * W],
                                         start=False, stop=True)
                        qT = small.tile([P, W], tdt, tag="qT")
                        nc.scalar.activation(out=qT[:], in_=p2t[:],
                                             func=mybir.ActivationFunctionType.Relu)
                        for h in range(2):
                            hw = w * 2 + h
                            p3 = psB.tile([P, P], f32, space="PSUM", tag="p3")
                            nc.tensor.matmul(out=p3[:],
                                             lhsT=qT[:, h * P:(h + 1) * P],
                                             rhs=wc2l_t[:], start=True, stop=False)
                            nc.tensor.matmul(out=p3[:],
                                             lhsT=ax[:, hw * P:(hw + 1) * P],
                                             rhs=wb2_t[:], start=False, stop=True)
                            nc.scalar.activation(
                                out=whg[:, hw, :], in_=p3[:],
                                func=mybir.ActivationFunctionType.Copy,
                                scale=rcpc_t[:, g * 8 + hw:g * 8 + hw + 1])
                    row0 = g * GR
                    nc.scalar.dma_start(
                        out=wh_tbl[row0:row0 + GR, :].rearrange(
                            "(w p) f -> p w f", p=P),
                        in_=whg[:])

                # ---------------- phase 2 (+ interleaved RS/finalize) --------
                def rs_start(cidx):
                    CHR, CHS = TROWS // RSC, SLICE // RSC
                    nc.gpsimd.collective_compute(
                        "ReduceScatter",
                        mybir.AluOpType.add,
                        replica_groups=[list(range(NCORES))],
                        ins=[t_tbl[cidx * CHR:(cidx + 1) * CHR, :]],
                        outs=[t_red[cidx * CHS:(cidx + 1) * CHS, :]],
                    )

                def fin_chunk(cidx):
                    CHS = SLICE // RSC
                    fin = finp.tile([P, WPC, P], tdt, tag="fin_in")
                    nc.scalar.dma_start(
                        out=fin[:],
                        in_=t_red[cidx * CHS:(cidx + 1) * CHS, :].rearrange(
                            "(w p) f -> p w f", p=P))
                    og = finp.tile([P, WPC, P], f32, tag="fin_out")
                    for w2 in range(WPC):
                        wabs = cidx * WPC + w2
                        nc.scalar.activation(out=og[:, w2, :], in_=fin[:, w2, :],
                                             func=mybir.ActivationFunctionType.Relu,
                                             scale=rlit_t[:, wabs:wabs + 1])
                    nc.scalar.dma_start(
                        out=out_e[cidx * CHS:(cidx + 1) * CHS, :].rearrange(
                            "(w p) f -> p w f", p=P),
                        in_=og[:])

                for g in range(NG2):
                    ccols, nm, nt, wb = sec2[g]
                    base = boff2[g]
                    bt = blobp.tile([P, BM2], i16, tag="b2")
                    nc.sync.dma_start(out=bt[:, 0:wb], in_=blob2_e[:, base:base + wb])
                    st = stage.tile([P, MC2, D], tdt, tag="st2")
                    col0 = 0
                    for c in range(NCH2):
                        seccols = ccols[c]
                        src = wh_tbl[b2[c]:b2[c + 1], :]
                        for o in range(0, seccols, CALL_COLS):
                            n = min(CALL_COLS, seccols - o)
                            nc.gpsimd.dma_gather(
                                out_ap=st[:, col0 + o:col0 + o + n, :],
                                in_ap=src,
                                idxs_ap=bt[:, (col0 + o) * 8:(col0 + o + n) * 8],
                                num_idxs=n * P,
                                num_idxs_reg=n * P,
                                elem_size=D,
                                queue_num=_next_q(),
                            )
                        col0 += seccols
                    relv = bt[:, nm * 8:nm * 8 + nm].bitcast(tdt)
                    mb = membp.tile([P, MC2, W], tdt, tag="mb2")
                    nc.vector.tensor_tensor(
                        out=mb[:, 0:nm, :],
                        in0=iota256_t[:, None, :].to_broadcast([P, nm, W]),
                        in1=relv.to_broadcast([P, nm, W]),
                        op=mybir.AluOpType.is_equal,
                    )
                    tg = small.tile([P, 2 * WPG, P], tdt, tag="tg")
                    for w in range(WPG):
                        cols = []
                        for c in range(NCH2):
                            o = sum(ccols[:c]) + int(np.sum(q2[g][:w, c]))
                            cols += [o + t for t in range(q2[g][w, c])]
                        for h in range(2):
                            if not cols:
                                nc.vector.memset(tg[:, w * 2 + h, :], 0.0)
                                continue
                            tacc = psB.tile([P, P], f32, space="PSUM", tag="tacc")
                            for i, col in enumerate(cols):
                                nc.tensor.matmul(
                                    out=tacc[:],
                                    lhsT=mb[:, col, h * P:(h + 1) * P],
                                    rhs=st[:, col, :],
                                    start=(i == 0), stop=(i == len(cols) - 1))
                            nc.scalar.copy(out=tg[:, w * 2 + h, :], in_=tacc[:])
                    row0 = g * GR
                    nc.scalar.dma_start(
                        out=t_tbl[row0:row0 + GR, :].rearrange(
                            "(w p) f -> p w f", p=P),
                        in_=tg[:])
                    if (g + 1) % GPC2 == 0:
                        cidx = (g + 1) // GPC2 - 1
                        rs_start(cidx)
                        if cidx >= 1:
                            fin_chunk(cidx - 1)
                fin_chunk(RSC - 1)

    nc.compile()
    return nc


# ----------------------------------------------------------------------------
# SPMD runner (jitted shard_map over the 8 NeuronCores, cached for reuse)
# ----------------------------------------------------------------------------

class SpmdRunner:
    def __init__(self, nc, n_cores):
        import jax
        import concourse.mybir as mybir
        from concourse.bass2jax import (
            _bass_exec_p, install_neuronx_cc_hook, partition_id_tensor)
        from jax.sharding import Mesh, PartitionSpec
        from jax.experimental.shard_map import shard_map

        install_neuronx_cc_hook()
        self.jax = jax
        self.n_cores = n_cores
        partition_name = nc.partition_id_tensor.name if nc.partition_id_tensor else None
        in_names, out_names, out_avals, zero_shapes = [], [], [], []
        for alloc in nc.m.functions[0].allocations:
            if not isinstance(alloc, mybir.MemoryLocationSet):
                continue
            name = alloc.memorylocations[0].name
            if alloc.kind == "ExternalInput":
                if name != partition_name:
                    in_names.append(name)
            elif alloc.kind == "ExternalOutput":
                out_names.append(name)
                shape = tuple(alloc.tensor_shape)
                dtype = mybir.dt.np(alloc.dtype)
                out_avals.append(jax.core.ShapedArray(shape, dtype))
                zero_shapes.append((shape, dtype))
        self.in_names, self.out_names = in_names, out_names
        self.out_avals, self.zero_shapes = out_avals, zero_shapes
        n_params, n_outs = len(in_names), len(out_avals)
        all_in_names = list(in_names) + list(out_names)
        if partition_name is not None:
            all_in_names.append(partition_name)

        def _body(*args):
            operands = list(args)
            if partition_name is not None:
                operands.append(partition_id_tensor())
            outs = _bass_exec_p.bind(
                *operands,
                out_avals=tuple(out_avals),
                in_names=tuple(all_in_names),
                out_names=tuple(out_names),
                lowering_input_output_aliases=(),
                sim_require_finite=True,
                sim_require_nnan=True,
                nc=nc,
            )
            return tuple(outs)

        devices = jax.devices()[:n_cores]
        self.mesh = Mesh(np.asarray(devices), ("core",))
        in_specs = (PartitionSpec("core"),) * (n_params + n_outs)
        out_specs = (PartitionSpec("core"),) * n_outs
        self.fn = jax.jit(
            shard_map(_body, mesh=self.mesh, in_specs=in_specs,
                      out_specs=out_specs, check_rep=False),
            keep_unused=True,
        )
        self._device_args = None
        self._pspec = PartitionSpec

    def put_inputs(self, in_maps):
        jax = self.jax
        n = self.n_cores
        sharding = jax.sharding.NamedSharding(self.mesh, self._pspec("core"))
        args = []
        for name in self.in_names:
            cat = np.concatenate([np.asarray(in_maps[c][name]) for c in range(n)], axis=0)
            args.append(jax.device_put(cat, sharding))
        for shape, dtype in self.zero_shapes:
            z = np.zeros((n * shape[0], *shape[1:]), dtype)
            args.append(jax.device_put(z, sharding))
        self._device_args = args
        jax.block_until_ready(args)

    def run(self):
        outs = self.fn(*self._device_args)
        self.jax.block_until_ready(outs)
        return outs

    def results(self, outs):
        n = self.n_cores
        res = []
        for c in range(n):
            d = {}
            for i, name in enumerate(self.out_names):
                shp = self.out_avals[i].shape
                d[name] = np.asarray(outs[i]).reshape(n, *shp)[c]
            res.append(d)
        return res


# ----------------------------------------------------------------------------
# public entry point
# ----------------------------------------------------------------------------

_CACHE = {}


def _get_runner(meta, reps):
    key = (repr(sorted(meta.items(), key=lambda kv: repr(kv[0]))), reps)
    if key not in _CACHE:
        nc = _build_nc(meta, reps=reps)
        _CACHE[key] = SpmdRunner(nc, NCORES)
    return _CACHE[key]


def assemble(meta, res):
    """Reassemble per-core RS-chunked output slices into the full table."""
    RSC, TROWS, SLICE = meta["RSC"], meta["TROWS"], meta["SLICE"]
    CHR, CHS = TROWS // RSC, SLICE // RSC
    full = np.empty((TROWS, D), np.float32)
    for c in range(RSC):
        for k in range(NCORES):
            full[c * CHR + k * CHS:c * CHR + (k + 1) * CHS] = \
                res[k]["out"][c * CHS:(c + 1) * CHS]
    return np.ascontiguousarray(full[:meta["n_lit"]])


def kernel(**inputs):
    meta, in_maps = _prep(inputs)
    r = _get_runner(meta, reps=1)
    r.put_inputs(in_maps)
    outs = r.run()
    res = r.results(outs)
    return assemble(meta, res)


# revision 3
# speedup vs baseline: 1.0634x; 1.0062x over previous
"""CNF GNN message-passing layer on 8 Trainium2 NeuronCores (Bass/Tile) — v2.

Architecture (edge parallel, clause-owner sharding, one-hot matmul seg-sum):
  - Core k owns clauses [k*CPC, (k+1)*CPC) and both message directions of
    their edges.
  - Windows are 256 rows (vs 128): seg-sum matmul rhs is the [slots, 256]
    one-hot, output accumulates in a [128, 256] fp32 PSUM tile.  Per-group
    quotas (max over the 8 cores only) cut gather padding to ~16-26%.
  - Phase 1 (l2c): gather raw lit rows per edge, seg-sum per clause window,
    then q = relu(sumT@W + deg*b); wh = rcp * (q@W2 + deg1*cf*W2[128] +
    deg1*valid*b2) — exact because rcp*deg1 == 1; the rcp scale rides the
    PSUM->SBUF copy (scalar activation Copy w/ per-partition scale).
  - The lit tail (rows >= 98304, beyond 3 int16 chunks) uses one group-wide
    column with a 1024-wide one-hot (4 matmul slices).
  - Phase 2 (c2l): gather wh rows per edge, seg-sum into lit windows ->
    partial table T_k, ReduceScatter(add) in RSC chunks interleaved with
    phase-2 groups, finalize mean (1/deg) + relu.
  - Per-group streams (gather idx + one-hot rel ids + rcp) are packed into a
    single int16 blob -> one sync DMA per group (+1 tiny aux DMA in p1).
"""
import sys
sys.path.insert(0, "/opt/trn_rl_repo")

import math
import numpy as np

P = 128
D = 128
NCORES = 8
W = 256            # window rows
WPG = 4            # windows per group
GR = W * WPG       # group rows (1024)
CH = 32768         # int16 chunk size
NQ = 4             # SWDGE queues
CALL_COLS = 8      # staging cols per dma_gather call (1024 idx HW cap)


def _ceil_to(x, m):
    return (x + m - 1) // m * m


def _wrap(idx16):
    """Flat slot-ordered idx array -> [128, n/16] wrapped+replicated layout."""
    n = idx16.size
    arr = idx16.reshape(n // 16, 16).T.reshape(16, -1)
    return np.tile(arr, (8, 1))


def _prep(inputs):
    lit_feat = np.asarray(inputs["lit_feat"], np.float32)
    clause_feat = np.asarray(inputs["clause_feat"], np.float32)
    el = np.asarray(inputs["edge_lit"]).astype(np.int64)
    ec = np.asarray(inputs["edge_clause"]).astype(np.int64)
    W_l2c = np.asarray(inputs["W_l2c"], np.float32)
    b_l2c = np.asarray(inputs["b_l2c"], np.float32)
    W_c2l = np.asarray(inputs["W_c2l"], np.float32)
    b_c2l = np.asarray(inputs["b_c2l"], np.float32)

    n_lit = lit_feat.shape[0]
    n_clause = clause_feat.shape[0]
    tdt = np.float16

    CPC = n_clause // NCORES
    CLROWS = _ceil_to(CPC, GR)
    NG1 = CLROWS // GR
    TROWS = _ceil_to(n_lit, GR)
    NG2 = TROWS // GR
    SLICE = TROWS // NCORES
    NW3 = SLICE // P
    RSC = 1
    for cand in (14, 7, 5, 4, 3, 2):
        if NW3 % cand == 0 and NG2 % cand == 0:
            RSC = cand
            break
    GPC2 = NG2 // RSC

    # phase-1 main chunks over lit rows + per-group tail column
    NCH1 = n_lit // CH                      # full chunks (3)
    TAIL0 = NCH1 * CH                       # 98304
    # phase-2 chunks over CLROWS
    b2 = list(range(0, CLROWS, CH)) + [CLROWS]
    NCH2 = len(b2) - 1

    degc = np.bincount(ec, minlength=n_clause).astype(np.float32)
    degl = np.bincount(el, minlength=n_lit).astype(np.float32)
    recipc = (1.0 / np.maximum(degc, 1.0)).astype(np.float32)

    owner = ec // CPC
    lc = ec - owner * CPC

    # ---- quotas (shared across cores: max over cores) ----
    g1 = lc // GR
    wl1 = (lc // W) % WPG
    ch1 = np.minimum(el // CH, NCH1)
    cnt1 = np.zeros((NCORES, NG1, WPG, NCH1 + 1), np.int64)
    np.add.at(cnt1, (owner, g1, wl1, ch1), 1)
    q1 = np.ceil(cnt1[:, :, :, :NCH1].max(axis=0) / P).astype(int)   # [NG1,WPG,NCH1]
    q1t = np.ceil(cnt1[:, :, :, NCH1].sum(axis=2).max(axis=0) / P).astype(int)  # [NG1]

    g2 = el // GR
    wl2 = (el // W) % WPG
    ch2 = lc // CH
    cnt2 = np.zeros((NCORES, NG2, WPG, NCH2), np.int64)
    np.add.at(cnt2, (owner, g2, wl2, ch2), 1)
    q2 = np.ceil(cnt2.max(axis=0) / P).astype(int)                   # [NG2,WPG,NCH2]

    # staging column layout per group: chunk-major for contiguous gather calls
    # p1: [c0 cols (w0..w3) | c1 | c2 | tail]; mb built over main cols only.
    def _layout1(g):
        off = {}
        col = 0
        for c in range(NCH1):
            for w in range(WPG):
                off[(w, c)] = col
                col += q1[g, w, c]
        moff = col                      # tail cols start here
        return off, moff, moff + q1t[g]

    def _layout2(g):
        off = {}
        col = 0
        for c in range(NCH2):
            for w in range(WPG):
                off[(w, c)] = col
                col += q2[g, w, c]
        return off, col

    sec1 = []    # per group: (colcnt per chunk, ncols main, ncols total, blob width)
    for g in range(NG1):
        ccols = [int(q1[g, :, c].sum()) for c in range(NCH1)] + [int(q1t[g])]
        nm = sum(ccols[:NCH1])
        nt = nm + ccols[NCH1]
        w_blob = nt * 8 + nm + ccols[NCH1] + 8   # idx + rel(main) + relt + rcp
        sec1.append((ccols, nm, nt, w_blob))
    sec2 = []
    for g in range(NG2):
        ccols = [int(q2[g, :, c].sum()) for c in range(NCH2)]
        nm = sum(ccols)
        w_blob = nm * 8 + nm
        sec2.append((ccols, nm, nm, w_blob))

    MC1 = max(s[2] for s in sec1)       # max total staging cols p1
    MM1 = max(s[1] for s in sec1)
    MT1 = max(int(x) for x in q1t)
    MC2 = max(s[2] for s in sec2)
    BM1 = max(s[3] for s in sec1)       # max blob width p1
    BM2 = max(s[3] for s in sec2)
    BW1 = sum(s[3] for s in sec1)       # total blob width p1
    BW2 = sum(s[3] for s in sec2)
    boff1 = np.cumsum([0] + [s[3] for s in sec1])
    boff2 = np.cumsum([0] + [s[3] for s in sec2])

    lit16 = np.ascontiguousarray(lit_feat.astype(tdt))

    in_maps = []
    for k in range(NCORES):
        m = owner == k
        elk, lck = el[m], lc[m]

        # ---------------- phase 1 streams ----------------
        blob1 = np.zeros((P, BW1), np.int16)
        kg1 = lck // GR
        kwl = (lck // W) % WPG
        kch = np.minimum(elk // CH, NCH1)
        order = np.lexsort((elk, kch, kwl, kg1))
        e_l, l_c = elk[order], lck[order]
        e_g, e_w, e_c = kg1[order], kwl[order], kch[order]
        # rank within (g, w, c)
        key = ((e_g * WPG + e_w) * (NCH1 + 1)) + e_c
        uniq, starts = np.unique(key, return_index=True)
        run_start = np.zeros(len(key), np.int64)
        run_start[starts] = starts
        run_start = np.maximum.accumulate(run_start)
        rank = np.arange(len(key)) - run_start

        for g in range(NG1):
            ccols, nm, nt, wb = sec1[g]
            off, moff, _ = _layout1(g)
            base = boff1[g]
            idx = np.zeros(nt * P, np.int16)
            rel = np.full((P, nm), -1.0, np.float16)
            relt = np.full((P, ccols[NCH1]), -1.0, np.float16)
            sel = e_g == g
            sl_l, sl_c = e_l[sel], l_c[sel]
            sl_w, sl_ch, sl_r = e_w[sel], e_c[sel], rank[sel]
            mn = sl_ch < NCH1
            # main slots
            colm = np.zeros(len(sl_l), np.int64)
            for w in range(WPG):
                for c in range(NCH1):
                    mm = (sl_w == w) & (sl_ch == c)
                    colm[mm] = off[(w, c)] + (sl_r[mm] >> 7)
            # tail slots: group-wide column(s), rank across the whole group
            tm = ~mn
            trank = np.cumsum(tm) - 1
            colm[tm] = moff + (trank[tm] >> 7)
            p = np.where(mn, sl_r & 127, trank[tm.cumsum() * 0 + 0] * 0)[
                :] if False else np.where(mn, sl_r & 127, 0)
            p[tm] = trank[tm] & 127
            slot = colm * P + p
            idx[slot] = np.where(mn, sl_l - sl_ch * CH, sl_l - TAIL0).astype(np.int16)
            rel[p[mn], colm[mn]] = (sl_c[mn] % W).astype(np.float16)
            relt[p[tm], colm[tm] - moff] = (sl_c[tm] - g * GR).astype(np.float16)

            blob1[:, base:base + nt * 8] = _wrap(idx)
            blob1[:, base + nt * 8:base + nt * 8 + nm] = rel.view(np.int16)
            o2 = base + nt * 8 + nm
            blob1[:, o2:o2 + ccols[NCH1]] = relt.view(np.int16)
            o3 = o2 + ccols[NCH1]
            cid = k * CPC + g * GR + np.arange(GR)
            cid = np.minimum(cid, n_clause - 1)
            rcp = recipc[cid].astype(np.float16).reshape(8, P).T  # [P, 8]
            blob1[:, o3:o3 + 8] = rcp.view(np.int16)

        rcpc = np.zeros((P, NG1 * 8), np.float32)
        for g in range(NG1):
            cid = k * CPC + g * GR + np.arange(GR)
            cid = np.minimum(cid, n_clause - 1)
            rcpc[:, g * 8:(g + 1) * 8] = recipc[cid].reshape(8, P).T

        # phase-1 aux rows [3, CLROWS]: deg, cf*deg1, valid*deg1
        cid = k * CPC + np.arange(CLROWS)
        valid = cid < n_clause
        cid = np.minimum(cid, n_clause - 1)
        dg = np.where(valid, degc[cid], 0.0)
        dg1 = np.maximum(dg, 1.0)
        degrow = dg[None, :].astype(np.float16)
        acf2 = np.stack([
            np.where(valid, clause_feat[cid, 0], 0.0) * dg1,
            valid.astype(np.float32) * dg1,
        ]).astype(np.float16)

        # ---------------- phase 2 streams ----------------
        blob2 = np.zeros((P, BW2), np.int16)
        kg2 = elk // GR
        kw2 = (elk // W) % WPG
        kc2 = lck // CH
        order2 = np.lexsort((lck, kc2, kw2, kg2))
        e2_l, l2_c = elk[order2], lck[order2]
        e2_g, e2_w, e2_c = kg2[order2], kw2[order2], kc2[order2]
        key2 = (e2_g * WPG + e2_w) * NCH2 + e2_c
        uniq2, starts2 = np.unique(key2, return_index=True)
        run2 = np.zeros(len(key2), np.int64)
        run2[starts2] = starts2
        run2 = np.maximum.accumulate(run2)
        rank2 = np.arange(len(key2)) - run2

        for g in range(NG2):
            ccols, nm, nt, wb = sec2[g]
            off, _ = _layout2(g)
            base = boff2[g]
            idx = np.zeros(nm * P, np.int16)
            rel = np.full((P, nm), -1.0, np.float16)
            sel = e2_g == g
            s_l, s_c = e2_l[sel], l2_c[sel]
            s_w, s_ch, s_r = e2_w[sel], e2_c[sel], rank2[sel]
            colm = np.zeros(len(s_l), np.int64)
            for w in range(WPG):
                for c in range(NCH2):
                    mm = (s_w == w) & (s_ch == c)
                    colm[mm] = off[(w, c)] + (s_r[mm] >> 7)
            p = s_r & 127
            slot = colm * P + p
            idx[slot] = (s_c - s_ch * CH).astype(np.int16)
            rel[p, colm] = (s_l % W).astype(np.float16)
            blob2[:, base:base + nm * 8] = _wrap(idx)
            blob2[:, base + nm * 8:base + nm * 8 + nm] = rel.view(np.int16)

        # finalize reciprocals, interleaved RS-chunk layout (as baseline)
        CHR, CHS = TROWS // RSC, SLICE // RSC
        w_all = np.arange(NW3)
        c_of_w = w_all // (NW3 // RSC)
        loc_of_w = w_all % (NW3 // RSC)
        rbase = c_of_w * CHR + k * CHS + loc_of_w * P
        lit_ids = rbase[:, None] + np.arange(P)[None, :]
        lvalid = lit_ids < n_lit
        lit_ids = np.minimum(lit_ids, n_lit - 1)
        rlit = np.where(lvalid, 1.0 / np.maximum(degl[lit_ids], 1.0), 1.0)
        rlit = rlit.astype(np.float32).T.copy()

        iota256 = np.broadcast_to(np.arange(W, dtype=tdt), (P, W)).copy()
        iota1k = np.broadcast_to(np.arange(GR, dtype=tdt), (P, GR)).copy()

        im = {
            "lit16": lit16,
            "blob1": blob1, "blob2": blob2,
            "degrow": degrow, "acf2": acf2, "rcpc": rcpc,
            "rlit": rlit, "iota256": iota256, "iota1k": iota1k,
            "wl2c": W_l2c.astype(tdt),
            "brow": b_l2c.astype(tdt)[None, :],
            "wc2l": W_c2l[:D].astype(tdt),
            "wb2": np.stack([W_c2l[D], b_c2l]).astype(tdt),
        }
        in_maps.append(im)

    meta = dict(
        n_lit=n_lit, n_clause=n_clause, CPC=CPC,
        NG1=NG1, CLROWS=CLROWS, NG2=NG2, TROWS=TROWS,
        SLICE=SLICE, NW3=NW3, RSC=RSC, GPC2=GPC2,
        NCH1=NCH1, TAIL0=TAIL0, NCH2=NCH2, b2=tuple(b2),
        q1=tuple(map(tuple, map(tuple, q1.reshape(NG1, -1)))),
        q1t=tuple(int(x) for x in q1t),
        q2=tuple(map(tuple, map(tuple, q2.reshape(NG2, -1)))),
        sec1=tuple((tuple(s[0]), s[1], s[2], s[3]) for s in sec1),
        sec2=tuple((tuple(s[0]), s[1], s[2], s[3]) for s in sec2),
        boff1=tuple(int(x) for x in boff1), boff2=tuple(int(x) for x in boff2),
        MC1=MC1, MM1=MM1, MT1=MT1, MC2=MC2, BW1=BW1, BW2=BW2,
        BM1=BM1, BM2=BM2,
    )
    return meta, in_maps


# ----------------------------------------------------------------------------
# bass program
# ----------------------------------------------------------------------------

def _build_nc(meta, reps=1):
    import concourse.bass as bass
    import concourse.bacc as bacc
    import concourse.mybir as mybir
    import concourse.tile as tile

    tdt = mybir.dt.float16
    f32 = mybir.dt.float32
    i16 = mybir.dt.int16

    n_lit = meta["n_lit"]
    NG1, NG2 = meta["NG1"], meta["NG2"]
    CLROWS, TROWS, SLICE, NW3 = meta["CLROWS"], meta["TROWS"], meta["SLICE"], meta["NW3"]
    RSC, GPC2 = meta["RSC"], meta["GPC2"]
    NCH1, TAIL0, NCH2 = meta["NCH1"], meta["TAIL0"], meta["NCH2"]
    b2 = meta["b2"]
    q1 = [np.array(q).reshape(WPG, NCH1) for q in meta["q1"]]
    q1t = meta["q1t"]
    q2 = [np.array(q).reshape(WPG, NCH2) for q in meta["q2"]]
    sec1, sec2 = meta["sec1"], meta["sec2"]
    boff1, boff2 = meta["boff1"], meta["boff2"]
    MC1, MM1, MT1, MC2 = meta["MC1"], meta["MM1"], meta["MT1"], meta["MC2"]
    BW1, BW2 = meta["BW1"], meta["BW2"]
    BM1, BM2 = meta["BM1"], meta["BM2"]
    WPC = NW3 // RSC

    nc = bacc.Bacc("TRN2", target_bir_lowering=False, debug=False,
                   num_devices=NCORES, num_swdge_queues=NQ)

    lit16 = nc.declare_dram_parameter("lit16", [n_lit, D], tdt, isOutput=False)
    blob1_e = nc.declare_dram_parameter("blob1", [P, BW1], i16, isOutput=False)
    blob2_e = nc.declare_dram_parameter("blob2", [P, BW2], i16, isOutput=False)
    degrow_e = nc.declare_dram_parameter("degrow", [1, CLROWS], tdt, isOutput=False)
    acf2_e = nc.declare_dram_parameter("acf2", [2, CLROWS], tdt, isOutput=False)
    rlit_e = nc.declare_dram_parameter("rlit", [P, NW3], f32, isOutput=False)
    rcpc_e = nc.declare_dram_parameter("rcpc", [P, NG1 * 8], f32, isOutput=False)
    iota256_e = nc.declare_dram_parameter("iota256", [P, W], tdt, isOutput=False)
    iota1k_e = nc.declare_dram_parameter("iota1k", [P, GR], tdt, isOutput=False)
    wl2c_e = nc.declare_dram_parameter("wl2c", [D, D], tdt, isOutput=False)
    brow_e = nc.declare_dram_parameter("brow", [1, D], tdt, isOutput=False)
    wc2l_e = nc.declare_dram_parameter("wc2l", [D, D], tdt, isOutput=False)
    wb2_e = nc.declare_dram_parameter("wb2", [2, D], tdt, isOutput=False)
    out_e = nc.declare_dram_parameter("out", [SLICE, D], f32, isOutput=True)

    wh_tbl = nc.dram_tensor("wh_tbl", [CLROWS, D], tdt)
    t_tbl = nc.dram_tensor("t_tbl", [TROWS, D], tdt)
    t_red = nc.dram_tensor("t_red", [SLICE, D], tdt)

    pool_dma_count = [0]

    def _next_q():
        q = pool_dma_count[0] % NQ
        pool_dma_count[0] += 1
        return q

    with tile.TileContext(nc) as tc:
        with (
            tc.tile_pool(name="const", bufs=1) as cpool,
            tc.tile_pool(name="stage", bufs=3) as stage,
            tc.tile_pool(name="memb", bufs=2) as membp,
            tc.tile_pool(name="blob", bufs=3) as blobp,
            tc.tile_pool(name="aux", bufs=2) as auxp,
            tc.tile_pool(name="small", bufs=3) as small,
            tc.tile_pool(name="fin", bufs=2) as finp,
            tc.tile_pool(name="psA", bufs=2, space="PSUM") as psA,
            tc.tile_pool(name="psB", bufs=2, space="PSUM") as psB,
        ):
            iota256_t = cpool.tile([P, W], tdt, tag="iota256")
            nc.sync.dma_start(out=iota256_t[:], in_=iota256_e[:, :])
            iota1k_t = cpool.tile([P, GR], tdt, tag="iota1k")
            nc.sync.dma_start(out=iota1k_t[:], in_=iota1k_e[:, :])
            wl2c_t = cpool.tile([D, D], tdt, tag="wl2c")
            nc.sync.dma_start(out=wl2c_t[:], in_=wl2c_e[:, :])
            brow_t = cpool.tile([1, D], tdt, tag="brow")
            nc.sync.dma_start(out=brow_t[:], in_=brow_e[:, :])
            wc2l_t = cpool.tile([D, D], tdt, tag="wc2l")
            nc.sync.dma_start(out=wc2l_t[:], in_=wc2l_e[:, :])
            wb2_t = cpool.tile([2, D], tdt, tag="wb2")
            nc.sync.dma_start(out=wb2_t[:], in_=wb2_e[:, :])
            rlit_t = cpool.tile([P, NW3], f32, tag="rlit")
            nc.sync.dma_start(out=rlit_t[:], in_=rlit_e[:, :])
            rcpc_t = cpool.tile([P, NG1 * 8], f32, tag="rcpc")
            nc.sync.dma_start(out=rcpc_t[:], in_=rcpc_e[:, :])

            for rep in range(reps):
                # ---------------- phase 1 ----------------
                for g in range(NG1):
                    ccols, nm, nt, wb = sec1[g]
                    base = boff1[g]
                    bt = blobp.tile([P, BM1], i16, tag="b1")
                    nc.sync.dma_start(out=bt[:, 0:wb], in_=blob1_e[:, base:base + wb])
                    st = stage.tile([P, MC1, D], tdt, tag="st1")
                    col0 = 0
                    for c in range(NCH1 + 1):
                        seccols = ccols[c]
                        if c < NCH1:
                            src = lit16[c * CH:min((c + 1) * CH, n_lit), :]
                        else:
                            src = lit16[TAIL0:n_lit, :]
                        for o in range(0, seccols, CALL_COLS):
                            n = min(CALL_COLS, seccols - o)
                            nc.gpsimd.dma_gather(
                                out_ap=st[:, col0 + o:col0 + o + n, :],
                                in_ap=src,
                                idxs_ap=bt[:, (col0 + o) * 8:(col0 + o + n) * 8],
                                num_idxs=n * P,
                                num_idxs_reg=n * P,
                                elem_size=D,
                                queue_num=_next_q(),
                            )
                        col0 += seccols
                    relv = bt[:, nt * 8:nt * 8 + nm].bitcast(tdt)
                    mb = membp.tile([P, MM1, W], tdt, tag="mb1")
                    nc.vector.tensor_tensor(
                        out=mb[:, 0:nm, :],
                        in0=iota256_t[:, None, :].to_broadcast([P, nm, W]),
                        in1=relv.to_broadcast([P, nm, W]),
                        op=mybir.AluOpType.is_equal,
                    )
                    qt = ccols[NCH1]
                    mbt = membp.tile([P, max(MT1, 1), GR], tdt, tag="mbt1")
                    if qt > 0:
                        reltv = bt[:, nt * 8 + nm:nt * 8 + nm + qt].bitcast(tdt)
                        nc.vector.tensor_tensor(
                            out=mbt[:, 0:qt, :],
                            in0=iota1k_t[:, None, :].to_broadcast([P, qt, GR]),
                            in1=reltv.to_broadcast([P, qt, GR]),
                            op=mybir.AluOpType.is_equal,
                        )
                    dgr = auxp.tile([1, GR], tdt, tag="dgr")
                    nc.sync.dma_start(out=dgr[:], in_=degrow_e[:, g * GR:(g + 1) * GR])
                    ax = auxp.tile([2, GR], tdt, tag="ax1")
                    nc.sync.dma_start(out=ax[:], in_=acf2_e[:, g * GR:(g + 1) * GR])

                    whg = small.tile([P, 2 * WPG, P], tdt, tag="whg")
                    for w in range(WPG):
                        acc = psA.tile([P, W], f32, space="PSUM", tag="acc1")
                        cols = []
                        for c in range(NCH1):
                            o = sum(ccols[:c]) + int(np.sum(q1[g][:w, c]))
                            cols += [o + t for t in range(q1[g][w, c])]
                        tcols = [nm + t for t in range(qt)]
                        ntot = len(cols) + len(tcols)
                        if ntot == 0:
                            nc.vector.memset(whg[:, w * 2, :], 0.0)
                            nc.vector.memset(whg[:, w * 2 + 1, :], 0.0)
                            continue
                        i = 0
                        for col in cols:
                            nc.tensor.matmul(out=acc[:], lhsT=st[:, col, :],
                                             rhs=mb[:, col, :],
                                             start=(i == 0), stop=(i == ntot - 1))
                            i += 1
                        for t in range(qt):
                            nc.tensor.matmul(
                                out=acc[:], lhsT=st[:, nm + t, :],
                                rhs=mbt[:, t, w * W:(w + 1) * W],
                                start=(i == 0), stop=(i == ntot - 1))
                            i += 1
                        accSB = small.tile([P, W], tdt, tag="accSB")
                        nc.vector.tensor_copy(out=accSB[:], in_=acc[:])
                        p2t = psA.tile([P, W], f32, space="PSUM", tag="p2t")
                        nc.tensor.matmul(out=p2t[:], lhsT=wl2c_t[:], rhs=accSB[:],
                                         start=True, stop=False)
                        nc.tensor.matmul(out=p2t[:], lhsT=brow_t[:],
                                         rhs=dgr[0:1, w * W:(w + 1) * W],
                                         start=False, stop=True)
                        qT = small.tile([P, W], tdt, tag="qT")
                        nc.scalar.activation(out=qT[:], in_=p2t[:],
                                             func=mybir.ActivationFunctionType.Relu)
                        for h in range(2):
                            hw = w * 2 + h
                            p3 = psB.tile([P, P], f32, space="PSUM", tag="p3")
                            nc.tensor.matmul(out=p3[:],
                                             lhsT=qT[:, h * P:(h + 1) * P],
                                             rhs=wc2l_t[:], start=True, stop=False)
                            nc.tensor.matmul(out=p3[:],
                                             lhsT=ax[:, hw * P:(hw + 1) * P],
                                             rhs=wb2_t[:], start=False, stop=True)
                            nc.scalar.activation(
                                out=whg[:, hw, :], in_=p3[:],
                                func=mybir.ActivationFunctionType.Copy,
                                scale=rcpc_t[:, g * 8 + hw:g * 8 + hw + 1])
                    row0 = g * GR
                    nc.scalar.dma_start(
                        out=wh_tbl[row0:row0 + GR, :].rearrange(
                            "(w p) f -> p w f", p=P),
                        in_=whg[:])

                # ---------------- phase 2 (+ interleaved RS/finalize) --------
                def rs_start(cidx):
                    CHR, CHS = TROWS // RSC, SLICE // RSC
                    nc.gpsimd.collective_compute(
                        "ReduceScatter",
                        mybir.AluOpType.add,
                        replica_groups=[list(range(NCORES))],
                        ins=[t_tbl[cidx * CHR:(cidx + 1) * CHR, :]],
                        outs=[t_red[cidx * CHS:(cidx + 1) * CHS, :]],
                    )

                def fin_chunk(cidx):
                    CHS = SLICE // RSC
                    fin = finp.tile([P, WPC, P], tdt, tag="fin_in")
                    nc.scalar.dma_start(
                        out=fin[:],
                        in_=t_red[cidx * CHS:(cidx + 1) * CHS, :].rearrange(
                            "(w p) f -> p w f", p=P))
                    og = finp.tile([P, WPC, P], f32, tag="fin_out")
                    for w2 in range(WPC):
                        wabs = cidx * WPC + w2
                        nc.scalar.activation(out=og[:, w2, :], in_=fin[:, w2, :],
                                             func=mybir.ActivationFunctionType.Relu,
                                             scale=rlit_t[:, wabs:wabs + 1])
                    nc.scalar.dma_start(
                        out=out_e[cidx * CHS:(cidx + 1) * CHS, :].rearrange(
                            "(w p) f -> p w f", p=P),
                        in_=og[:])

                for g in range(NG2):
                    ccols, nm, nt, wb = sec2[g]
                    base = boff2[g]
                    bt = blobp.tile([P, BM2], i16, tag="b2")
                    nc.sync.dma_start(out=bt[:, 0:wb], in_=blob2_e[:, base:base + wb])
                    st = stage.tile([P, MC2, D], tdt, tag="st2")
                    col0 = 0
                    for c in range(NCH2):
                        seccols = ccols[c]
                        src = wh_tbl[b2[c]:b2[c + 1], :]
                        for o in range(0, seccols, CALL_COLS):
                            n = min(CALL_COLS, seccols - o)
                            nc.gpsimd.dma_gather(
                                out_ap=st[:, col0 + o:col0 + o + n, :],
                                in_ap=src,
                                idxs_ap=bt[:, (col0 + o) * 8:(col0 + o + n) * 8],
                                num_idxs=n * P,
                                num_idxs_reg=n * P,
                                elem_size=D,
                                queue_num=_next_q(),
                            )
                        col0 += seccols
                    relv = bt[:, nm * 8:nm * 8 + nm].bitcast(tdt)
                    mb = membp.tile([P, MC2, W], tdt, tag="mb2")
                    nc.vector.tensor_tensor(
                        out=mb[:, 0:nm, :],
                        in0=iota256_t[:, None, :].to_broadcast([P, nm, W]),
                        in1=relv.to_broadcast([P, nm, W]),
                        op=mybir.AluOpType.is_equal,
                    )
                    tg = small.tile([P, 2 * WPG, P], tdt, tag="tg")
                    for w in range(WPG):
                        cols = []
                        for c in range(NCH2):
                            o = sum(ccols[:c]) + int(np.sum(q2[g][:w, c]))
                            cols += [o + t for t in range(q2[g][w, c])]
                        for h in range(2):
                            if not cols:
                                nc.vector.memset(tg[:, w * 2 + h, :], 0.0)
                                continue
                            tacc = psB.tile([P, P], f32, space="PSUM", tag="tacc")
                            for i, col in enumerate(cols):
                                nc.tensor.matmul(
                                    out=tacc[:],
                                    lhsT=mb[:, col, h * P:(h + 1) * P],
                                    rhs=st[:, col, :],
                                    start=(i == 0), stop=(i == len(cols) - 1))
                            nc.scalar.copy(out=tg[:, w * 2 + h, :], in_=tacc[:])
                    row0 = g * GR
                    nc.scalar.dma_start(
                        out=t_tbl[row0:row0 + GR, :].rearrange(
                            "(w p) f -> p w f", p=P),
                        in_=tg[:])
                    if (g + 1) % GPC2 == 0:
                        cidx = (g + 1) // GPC2 - 1
                        rs_start(cidx)
                        if cidx >= 1:
                            fin_chunk(cidx - 1)
                fin_chunk(RSC - 1)

    nc.compile()
    return nc


# ----------------------------------------------------------------------------
# SPMD runner (jitted shard_map over the 8 NeuronCores, cached for reuse)
# ----------------------------------------------------------------------------

class SpmdRunner:
    def __init__(self, nc, n_cores):
        import jax
        import concourse.mybir as mybir
        from concourse.bass2jax import (
            _bass_exec_p, install_neuronx_cc_hook, partition_id_tensor)
        from jax.sharding import Mesh, PartitionSpec
        from jax.experimental.shard_map import shard_map

        install_neuronx_cc_hook()
        self.jax = jax
        self.n_cores = n_cores
        partition_name = nc.partition_id_tensor.name if nc.partition_id_tensor else None
        in_names, out_names, out_avals, zero_shapes = [], [], [], []
        for alloc in nc.m.functions[0].allocations:
            if not isinstance(alloc, mybir.MemoryLocationSet):
                continue
            name = alloc.memorylocations[0].name
            if alloc.kind == "ExternalInput":
                if name != partition_name:
                    in_names.append(name)
            elif alloc.kind == "ExternalOutput":
                out_names.append(name)
                shape = tuple(alloc.tensor_shape)
                dtype = mybir.dt.np(alloc.dtype)
                out_avals.append(jax.core.ShapedArray(shape, dtype))
                zero_shapes.append((shape, dtype))
        self.in_names, self.out_names = in_names, out_names
        self.out_avals, self.zero_shapes = out_avals, zero_shapes
        n_params, n_outs = len(in_names), len(out_avals)
        all_in_names = list(in_names) + list(out_names)
        if partition_name is not None:
            all_in_names.append(partition_name)

        def _body(*args):
            operands = list(args)
            if partition_name is not None:
                operands.append(partition_id_tensor())
            outs = _bass_exec_p.bind(
                *operands,
                out_avals=tuple(out_avals),
                in_names=tuple(all_in_names),
                out_names=tuple(out_names),
                lowering_input_output_aliases=(),
                sim_require_finite=True,
                sim_require_nnan=True,
                nc=nc,
            )
            return tuple(outs)

        devices = jax.devices()[:n_cores]
        self.mesh = Mesh(np.asarray(devices), ("core",))
        in_specs = (PartitionSpec("core"),) * (n_params + n_outs)
        out_specs = (PartitionSpec("core"),) * n_outs
        self.fn = jax.jit(
            shard_map(_body, mesh=self.mesh, in_specs=in_specs,
                      out_specs=out_specs, check_rep=False),
            keep_unused=True,
        )
        self._device_args = None
        self._pspec = PartitionSpec

    def put_inputs(self, in_maps):
        jax = self.jax
        n = self.n_cores
        sharding = jax.sharding.NamedSharding(self.mesh, self._pspec("core"))
        args = []
        for name in self.in_names:
            cat = np.concatenate([np.asarray(in_maps[c][name]) for c in range(n)], axis=0)
            args.append(jax.device_put(cat, sharding))
        for shape, dtype in self.zero_shapes:
            z = np.zeros((n * shape[0], *shape[1:]), dtype)
            args.append(jax.device_put(z, sharding))
        self._device_args = args
        jax.block_until_ready(args)

    def run(self):
        outs = self.fn(*self._device_args)
        self.jax.block_until_ready(outs)
        return outs

    def results(self, outs):
        n = self.n_cores
        res = []
        for c in range(n):
            d = {}
            for i, name in enumerate(self.out_names):
                shp = self.out_avals[i].shape
                d[name] = np.asarray(outs[i]).reshape(n, *shp)[c]
            res.append(d)
        return res


# ----------------------------------------------------------------------------
# public entry point
# ----------------------------------------------------------------------------

_CACHE = {}


def _get_runner(meta, reps):
    key = (repr(sorted(meta.items(), key=lambda kv: repr(kv[0]))), reps)
    if key not in _CACHE:
        nc = _build_nc(meta, reps=reps)
        _CACHE[key] = SpmdRunner(nc, NCORES)
    return _CACHE[key]


def assemble(meta, res):
    """Reassemble per-core RS-chunked output slices into the full table."""
    RSC, TROWS, SLICE = meta["RSC"], meta["TROWS"], meta["SLICE"]
    CHR, CHS = TROWS // RSC, SLICE // RSC
    full = np.empty((TROWS, D), np.float32)
    for c in range(RSC):
        for k in range(NCORES):
            full[c * CHR + k * CHS:c * CHR + (k + 1) * CHS] = \
                res[k]["out"][c * CHS:(c + 1) * CHS]
    return np.ascontiguousarray(full[:meta["n_lit"]])


def kernel(**inputs):
    meta, in_maps = _prep(inputs)
    r = _get_runner(meta, reps=1)
    r.put_inputs(in_maps)
    outs = r.run()
    res = r.results(outs)
    return assemble(meta, res)
